# revision 1
# baseline (speedup 1.0000x reference)
"""Trainium2 Bass kernel for nn_BuzCusStructureSim (dense_transformer).

Exact math simplifications (hold for ANY input values):
 - softmax over a trailing size-1 axis is exactly 1.0, so the _weighted_sum
   calls are plain sums over the trailing feature axis, and the final W_f
   mixing reduces to out = BS_out + CS_out (W_bs/W_cs/W_f never matter).
 - the attention softmax (query axis s) is immediately contracted with
   Bt[s]:  BR[t,h] = (sum_s Bt[s] e[s,t]) / (sum_s e[s,t]),
   e = exp(scores/16); both sums come from one PE matmul, lhsT = [Bt | 1].
 - Q K^T = E (W1 W2^T) E^T: M_h = W1_h W2_h^T precomputed once per core.

Sharding: data-parallel over batch (16/8 = 2 per core).
Matmul dtype float32r (TF32-ish, ~1.6e-4 elementwise, full PE rate).

This environment is instruction-fetch bound (~60-120 MB/s instruction
streaming: ~1.2us per unrolled matmul vs ~240ns inside a HW loop), so all
hot code runs inside tc.For_i loops with IRAM-resident bodies and dynamic
(register-offset) access patterns.  Matmul stationary operands must be
compile-time static, so per-head weights are staged into fixed SBUF
buffers by an SBUF->SBUF DMA each iteration.
"""

import numpy as np

import concourse.bacc as bacc
import concourse.tile as tile
from concourse import mybir
from concourse.bass import ds
from concourse.bass_utils import run_bass_kernel_spmd

B, S, D, H, FB, K, FC = 16, 512, 256, 8, 128, 64, 32
NCORES = 8
BL = B // NCORES
NT = S // 128     # 4
ND = D // 128     # 2

F32 = mybir.dt.float32
F32R = mybir.dt.float32r
AX = mybir.AxisListType
ALU = mybir.AluOpType
ACT = mybir.ActivationFunctionType
SCALE = 1.0 / 16.0
REPEAT = 1
HREP = 1
WREP = 1
DEBUG = False
NO_C = False
NO_EXP = False
NO_ND = False


def build():
    nc = bacc.Bacc("TRN2")
    io = {}
    io["b_t"] = nc.dram_tensor("b_target", [BL, S, FB], F32, kind="ExternalInput")
    io["b_i"] = nc.dram_tensor("b_infected", [BL, S, FB], F32, kind="ExternalInput")
    io["e_t"] = nc.dram_tensor("e_target", [BL, S, D], F32, kind="ExternalInput")
    io["e_i"] = nc.dram_tensor("e_infected", [BL, S, D], F32, kind="ExternalInput")
    io["c_t"] = nc.dram_tensor("c_target", [BL, S, K, FC], F32, kind="ExternalInput")
    io["c_i"] = nc.dram_tensor("c_infected", [BL, S, K, FC], F32, kind="ExternalInput")
    io["w1"] = nc.dram_tensor("w1", [H, D, D], F32, kind="ExternalInput")
    io["w2"] = nc.dram_tensor("w2", [H, D, D], F32, kind="ExternalInput")
    io["gbs"] = nc.dram_tensor("gamma_bs", [S], F32, kind="ExternalInput")
    io["bbs"] = nc.dram_tensor("beta_bs", [S], F32, kind="ExternalInput")
    io["gcs"] = nc.dram_tensor("gamma_cs", [S], F32, kind="ExternalInput")
    io["bcs"] = nc.dram_tensor("beta_cs", [S], F32, kind="ExternalInput")
    io["o_out"] = nc.dram_tensor("o_out", [BL, S], F32, kind="ExternalOutput")
    io["o_bs"] = nc.dram_tensor("o_bs", [BL, S], F32, kind="ExternalOutput")
    io["o_cs"] = nc.dram_tensor("o_cs", [BL, S], F32, kind="ExternalOutput")
    io["ident"] = nc.inline_tensor(np.eye(128, dtype=np.float32), name="ident")
    if DEBUG:
        io["dbg_m"] = nc.dram_tensor("dbg_m", [128, H, ND, D], F32, kind="ExternalOutput")
        io["dbg_et"] = nc.dram_tensor("dbg_et", [128, 2 * BL, ND, S], F32, kind="ExternalOutput")
        io["dbg_bto"] = nc.dram_tensor("dbg_bto", [128, 2 * BL, NT, 2], F32, kind="ExternalOutput")
        io["dbg_nd"] = nc.dram_tensor("dbg_nd", [2, BL, 2 * H, S], F32, kind="ExternalOutput")

    with tile.TileContext(nc) as tc:
        _emit(nc, tc, io)
    nc.compile()
    return nc


def _emit(nc, tc, io):
    from contextlib import ExitStack

    with ExitStack() as ctx:
        const = ctx.enter_context(tc.tile_pool(name="const", bufs=1))
        big = ctx.enter_context(tc.tile_pool(name="big", bufs=1))
        stage = ctx.enter_context(tc.tile_pool(name="stage", bufs=1))
        sm = ctx.enter_context(tc.tile_pool(name="sm", bufs=1))
        trps = ctx.enter_context(tc.tile_pool(name="trps", bufs=1, space="PSUM"))
        gp = ctx.enter_context(tc.tile_pool(name="gp", bufs=1, space="PSUM"))
        scp = ctx.enter_context(tc.tile_pool(name="scp", bufs=1, space="PSUM"))
        ndp = ctx.enter_context(tc.tile_pool(name="ndp", bufs=1, space="PSUM"))

        # ---- constants ----
        ident = const.tile([128, 128], F32)
        nc.sync.dma_start(ident[:], io["ident"][:])
        ones_col = const.tile([128, 1], F32)
        nc.vector.memset(ones_col[:], 1.0)
        ones_row = const.tile([1, 128], F32)
        nc.vector.memset(ones_row[:], 1.0)
        eps_t = const.tile([1, 1], F32)
        nc.vector.memset(eps_t[:], 1e-16)

        def ln_vec(name, neg):
            tl = const.tile([128, NT], F32, tag=name, name=name)
            nc.sync.dma_start(
                tl[:], io[name].ap().rearrange("(hi lo) -> lo hi", lo=128))
            if neg:
                nc.vector.tensor_scalar_mul(tl[:], tl[:], -1.0)
            return tl

        gbs_t = ln_vec("gbs", True)   # negated: folds cosine minus sign
        bbs_t = ln_vec("bbs", False)
        gcs_t = ln_vec("gcs", False)
        bcs_t = ln_vec("bcs", False)

        # ---- persistent buffers ----
        m_all = big.tile([128, H, ND, D], F32R)         # M_h  [d_lo,(h,dd),dp]
        et_all = big.tile([128, 2 * BL, ND, S], F32R)   # E^T  [d_lo,(bt,dd),s]
        bto_all = big.tile([128, 2 * BL, NT, 2], F32R)  # [Bt | 1] lhsT
        ct_all = big.tile([128, BL, 2, NT, K], F32)     # C inner sums
        if NO_C:
            nc.vector.memset(ct_all[:], 0.5)
        nd2 = big.tile([2, BL, 2 * H, S], F32)          # num/den rows
        btf = big.tile([128, NT, 2], F32)               # staging for bto
        nc.vector.memset(btf[:, :, 1:2], 1.0)

        w1r = io["w1"].ap().rearrange("h (dd p) e -> h dd p e", p=128)
        w2r = io["w2"].ap().rearrange("h (dd p) e -> h dd p e", p=128)
        ers = [io["e_t"].ap().rearrange("b (st p) d -> b st p d", p=128),
               io["e_i"].ap().rearrange("b (st p) d -> b st p d", p=128)]
        brs = [io["b_t"].ap().rearrange("b (st p) f -> b st p f", p=128),
               io["b_i"].ap().rearrange("b (st p) f -> b st p f", p=128)]
        crs = [io["c_t"].ap().rearrange("b (st p) k f -> b st p k f", p=128),
               io["c_i"].ap().rearrange("b (st p) k f -> b st p k f", p=128)]

        for _rep in range(REPEAT):
            # ============ W loop: M_h = W1_h @ W2_h^T ============
            with tc.For_i(0, H * WREP, 1) as hw_raw:
                hw = hw_raw % H if WREP > 1 else hw_raw
                wst = [stage.tile([128, ND, D], F32, tag=f"wst{w}",
                                  name=f"wst{w}") for w in range(2)]
                for w, wr in enumerate((w1r, w2r)):
                    for dd in range(ND):
                        nc.sync.dma_start(
                            wst[w][:, dd, :],
                            wr[ds(hw, 1), dd, :, :].squeeze(0))
                wts = []
                for w in range(2):
                    ptr = trps.tile([128, ND, D], F32, tag="tr",
                                    name=f"ptrw{w}")
                    for dd in range(ND):
                        for ee in range(ND):
                            nc.tensor.transpose(
                                ptr[:, ee, dd * 128:(dd + 1) * 128],
                                wst[w][:, dd, ee * 128:(ee + 1) * 128],
                                ident[:])
                    wtr = stage.tile([128, ND, D], F32R, tag=f"wtr{w}",
                                     name=f"wtr{w}")
                    nc.vector.tensor_copy(wtr[:], ptr[:])
                    wts.append(wtr)
                pm = gp.tile([128, ND, D], F32, tag="g", name="pmw")
                for dt_ in range(ND):
                    for ee in range(ND):
                        nc.tensor.matmul(
                            pm[:, dt_, :],
                            wts[0][:, ee, dt_ * 128:(dt_ + 1) * 128],
                            wts[1][:, ee, :],
                            start=(ee == 0), stop=(ee == ND - 1))
                nc.vector.tensor_copy(
                    m_all[:, ds(hw, 1), :, :].squeeze(1), pm[:])

            # ============ E loop: E^T and Bt ============
            with tc.For_i(0, BL, 1) as bv:
                for ti in range(2):
                    est = stage.tile([128, NT, D], F32, tag=f"est{ti}",
                                     name=f"est{ti}")
                    nc.sync.dma_start(
                        est[:],
                        ers[ti][ds(bv, 1), :, :, :].squeeze(0)
                        .transpose([1, 0, 2]))
                    for dd in range(ND):
                        ptr = trps.tile([128, S], F32, tag="tr",
                                        name=f"ptre{ti}{dd}")
                        for st in range(NT):
                            nc.tensor.transpose(
                                ptr[:, st * 128:(st + 1) * 128],
                                est[:, st, dd * 128:(dd + 1) * 128],
                                ident[:])
                        nc.vector.tensor_copy(
                            et_all[:, ds(2 * bv + ti, 1), dd, :].squeeze(1),
                            ptr[:])
                    bst = stage.tile([128, NT, FB], F32, tag=f"bst{ti}",
                                     name=f"bst{ti}")
                    nc.sync.dma_start(
                        bst[:],
                        brs[ti][ds(bv, 1), :, :, :].squeeze(0)
                        .transpose([1, 0, 2]))
                    nc.vector.reduce_sum(btf[:, :, 0:1], bst[:], axis=AX.X)
                    nc.vector.tensor_copy(
                        bto_all[:, ds(2 * bv + ti, 1), :, :].squeeze(1),
                        btf[:])

            # ============ head loops (one For_i per bt) ============
            for bt in [x for _ in range(HREP) for x in range(2 * BL)]:
                b, ti = bt // 2, bt % 2
                et = et_all[:, bt, :, :]
                bto = bto_all[:, bt, :, :]
                with tc.For_i(0, H // 2, 1) as ii:
                    for half in range(2):
                        hx = 2 * ii + half
                        sfx = f"{half}"
                        mst = stage.tile([128, ND, D], F32R, tag=f"mst{sfx}",
                                         name=f"mst{sfx}")
                        nc.vector.tensor_copy(
                            mst[:], m_all[:, ds(hx, 1), :, :].squeeze(1))
                        gps = gp.tile([128, ND, S], F32, tag="g", name="gps")
                        for dtp in range(ND):
                            for dd in range(ND):
                                nc.tensor.matmul(
                                    gps[:, dtp, :],
                                    mst[:, dd, dtp * 128:(dtp + 1) * 128],
                                    et[:, dd, :],
                                    start=(dd == 0), stop=(dd == ND - 1))
                        gsb = sm.tile([128, ND, S], F32R, tag=f"gsb{sfx}",
                                      name=f"gsb{sfx}")
                        nc.vector.tensor_copy(gsb[:], gps[:])
                        e1 = sm.tile([128, NT, S], F32R, tag=f"e1{sfx}",
                                     name=f"e1{sfx}")
                        scps = scp.tile([128, 2, S], F32, tag=f"sc{sfx}",
                                        name=f"scps{sfx}")
                        for jh in range(2):
                            for j in range(2):
                                st = 2 * jh + j
                                for dtp in range(ND):
                                    nc.tensor.matmul(
                                        scps[:, j, :],
                                        gsb[:, dtp, st * 128:(st + 1) * 128],
                                        et[:, dtp, :],
                                        start=(dtp == 0),
                                        stop=(dtp == ND - 1))
                            nc.scalar.activation(
                                e1[:, 2 * jh:2 * jh + 2, :], scps[:],
                                ACT.Exp, scale=SCALE)
                        ndps = ndp.tile([2, S], F32, tag="nd", name="ndps")
                        for st in range(NT):
                            nc.tensor.matmul(
                                ndps[:], bto[:, st, :], e1[:, st, :],
                                start=(st == 0), stop=(st == NT - 1))
                        nc.vector.tensor_copy(
                            nd2[:, b, ds(ti * H + hx, 1), :].squeeze(1),
                            ndps[:])
                        if NO_C:
                            continue
                        # C branch: one k-slab per half-iteration
                        cst = stage.tile([128, NT, 8, FC], F32,
                                         tag=f"cst{sfx}", name=f"cst{sfx}")
                        nc.sync.dma_start(
                            cst[:],
                            crs[ti][b].transpose([1, 0, 2, 3])
                            [:, :, ds(8 * hx, 8), :])
                        red = stage.tile([128, NT, 8], F32, tag=f"crd{sfx}",
                                         name=f"crd{sfx}")
                        nc.vector.reduce_sum(red[:], cst[:], axis=AX.X)
                        nc.vector.tensor_copy(
                            ct_all[:, b, ti, :, ds(8 * hx, 8)], red[:])

            if DEBUG:
                nc.sync.dma_start(io["dbg_m"][:], m_all[:].bitcast(F32))
                nc.sync.dma_start(io["dbg_et"][:], et_all[:].bitcast(F32))
                nc.sync.dma_start(io["dbg_bto"][:], bto_all[:].bitcast(F32))
                nc.sync.dma_start(io["dbg_nd"][:], nd2[:])

            # ============ tail loop over b ============
            with tc.For_i(0, BL, 1) as bv:
                ndst = sm.tile([2 * H * 2, S], F32, tag="ndst", name="ndst")
                nc.sync.dma_start(
                    ndst[:], nd2[:, ds(bv, 1), :, :].squeeze(1))
                ndtr = trps.tile([128, 128], F32, tag="tr", name="ndtr")
                for tt in range(NT):
                    nc.tensor.transpose(
                        ndtr[:, tt * 32:(tt + 1) * 32],
                        ndst[:, tt * 128:(tt + 1) * 128], ident[0:32, 0:32])
                brnd = sm.tile([128, NT, 2 * 2 * H], F32, tag="brnd", name="brnd")
                nc.vector.tensor_copy(
                    brnd[:].rearrange("p a h -> p (a h)"), ndtr[:])
                # per tt: 32 cols = [num(16) | den(16)], hh = ti*8 + h
                rec = sm.tile([128, NT, 2 * H], F32, tag="rec", name="rec")
                brm = sm.tile([128, NT, 2 * H], F32, tag="brm", name="brm")
                nc.vector.reciprocal(rec[:], brnd[:, :, 16:32])
                nc.vector.tensor_mul(brm[:], brnd[:, :, 0:16], rec[:])
                a1 = brm[:, :, 0:H]
                a2 = brm[:, :, H:2 * H]
                pr = sm.tile([128, NT, H], F32, tag="pr", name="pr")
                red3 = sm.tile([128, NT, 3], F32, tag="red3", name="red3")
                nc.vector.tensor_mul(pr[:], a1, a2)
                nc.vector.reduce_sum(red3[:, :, 0:1], pr[:], axis=AX.X)
                nc.vector.tensor_mul(pr[:], a1, a1)
                nc.vector.reduce_sum(red3[:, :, 1:2], pr[:], axis=AX.X)
                nc.vector.tensor_mul(pr[:], a2, a2)
                nc.vector.reduce_sum(red3[:, :, 2:3], pr[:], axis=AX.X)
                nc.vector.tensor_scalar_max(
                    red3[:, :, 1:3], red3[:, :, 1:3], 1e-12)
                m4 = sm.tile([128, NT], F32, tag="m4", name="m4")
                nc.vector.tensor_mul(m4[:], red3[:, :, 1], red3[:, :, 2])
                nc.scalar.activation(m4[:], m4[:], ACT.Ln)
                nc.scalar.activation(m4[:], m4[:], ACT.Exp, scale=-0.5)
                lnin = sm.tile([128, 16], F32, tag="lnin", name="lnin")
                nc.vector.tensor_mul(
                    lnin[:, 0:4], red3[:, :, 0], m4[:])  # +dot*rstd = -cos
                # ---- C tail ----
                ctb = ct_all[:, ds(bv, 1), 0, :, :].squeeze(1)
                cib = ct_all[:, ds(bv, 1), 1, :, :].squeeze(1)
                cm = sm.tile([128, NT, K], F32, tag="cm", name="cm")
                nc.vector.tensor_add(cm[:], ctb, cib)
                nc.vector.tensor_scalar_mul(cm[:], cm[:], 0.5)
                yt = sm.tile([128, NT, K], F32, tag="yt", name="yt")
                yi = sm.tile([128, NT, K], F32, tag="yi", name="yi")
                nc.vector.tensor_scalar(
                    yt[:], ctb, 1e-7, 1.0, op0=ALU.max, op1=ALU.min)
                nc.vector.tensor_scalar(
                    yi[:], cib, 1e-7, 1.0, op0=ALU.max, op1=ALU.min)
                nc.vector.tensor_scalar(
                    cm[:], cm[:], 1e-7, 1.0, op0=ALU.max, op1=ALU.min)
                lt = sm.tile([128, NT, K], F32, tag="lt", name="lt")
                li = sm.tile([128, NT, K], F32, tag="li", name="li")
                lm = sm.tile([128, NT, K], F32, tag="lm", name="lm")
                nc.scalar.activation(lt[:], yt[:], ACT.Ln)
                nc.scalar.activation(li[:], yi[:], ACT.Ln)
                nc.scalar.activation(lm[:], cm[:], ACT.Ln)
                nc.vector.tensor_sub(lt[:], lt[:], lm[:])
                nc.vector.tensor_mul(lt[:], lt[:], yt[:])
                nc.vector.tensor_sub(li[:], li[:], lm[:])
                nc.vector.tensor_mul(li[:], li[:], yi[:])
                nc.vector.tensor_add(lt[:], lt[:], li[:])
                nc.vector.reduce_sum(lnin[:, 4:8], lt[:], axis=AX.X)
                nc.vector.tensor_scalar_mul(lnin[:, 4:8], lnin[:, 4:8], 0.5)
                # ---- combined LN for BS (cols 0:4) and CS (cols 4:8) ----
                nc.vector.tensor_mul(lnin[:, 8:16], lnin[:, 0:8],
                                     lnin[:, 0:8])
                ps_s = trps.tile([1, 16], F32, tag="tr", name="ps_s")
                nc.tensor.matmul(ps_s[:], ones_col[:], lnin[:])
                ss = sm.tile([1, 16], F32, tag="ss", name="ss")
                nc.vector.tensor_copy(ss[:], ps_s[:])
                s4 = sm.tile([1, 4], F32, tag="s4", name="s4")
                nc.vector.reduce_sum(
                    s4[:], ss[:].rearrange("p (g c) -> p g c", c=4),
                    axis=AX.X)  # [Scos, Scv, Scos2, Scv2]
                mr = sm.tile([1, 4], F32, tag="mr", name="mr")
                nc.vector.tensor_scalar_mul(mr[0:1, 0:2], s4[0:1, 0:2],
                                            1.0 / S)
                st2 = sm.tile([1, 2], F32, tag="st2", name="st2")
                nc.vector.tensor_scalar_mul(st2[:], s4[0:1, 2:4], 1.0 / S)
                msq = sm.tile([1, 2], F32, tag="msq", name="msq")
                nc.vector.tensor_mul(msq[:], mr[0:1, 0:2], mr[0:1, 0:2])
                nc.vector.tensor_sub(st2[:], st2[:], msq[:])  # var
                nc.scalar.activation(st2[:], st2[:], ACT.Ln,
                                     bias=eps_t[0:1, :])
                nc.scalar.activation(mr[0:1, 2:4], st2[:], ACT.Exp,
                                     scale=-0.5)
                bc_ps = trps.tile([128, 4], F32, tag="tr", name="bc_ps")
                nc.tensor.matmul(bc_ps[:], ones_row[:], mr[:])
                bc = sm.tile([128, 4], F32, tag="bc", name="bc")
                nc.vector.tensor_copy(bc[:], bc_ps[:])
                outs3 = sm.tile([128, 3, NT], F32, tag="outs3", name="outs3")
                # BS
                xm = sm.tile([128, NT], F32, tag="xm", name="xm")
                nc.vector.tensor_scalar_sub(xm[:], lnin[:, 0:4], bc[:, 0:1])
                nc.vector.tensor_scalar_mul(xm[:], xm[:], bc[:, 2:3])
                nc.vector.tensor_mul(xm[:], xm[:], gbs_t[:])
                nc.vector.tensor_add(outs3[:, 1, :], xm[:], bbs_t[:])
                # CS
                nc.vector.tensor_scalar_sub(xm[:], lnin[:, 4:8], bc[:, 1:2])
                nc.vector.tensor_scalar_mul(xm[:], xm[:], bc[:, 3:4])
                nc.vector.tensor_mul(xm[:], xm[:], gcs_t[:])
                nc.vector.tensor_add(outs3[:, 2, :], xm[:], bcs_t[:])
                nc.vector.tensor_add(outs3[:, 0, :], outs3[:, 1, :],
                                     outs3[:, 2, :])
                for oi, od in enumerate((io["o_out"], io["o_bs"],
                                         io["o_cs"])):
                    for tt in range(NT):
                        nc.sync.dma_start(
                            od[ds(bv, 1), tt * 128:(tt + 1) * 128]
                            .squeeze(0),
                            outs3[:, oi, tt])


_NC_CACHE = []
TRACE = False
LAST_RESULT = []
ABLATE = set()


def kernel(**inputs):
    if not _NC_CACHE:
        _NC_CACHE.append(build())
    nc = _NC_CACHE[0]

    def shard(x, i):
        return np.ascontiguousarray(x[i * BL:(i + 1) * BL])

    in_maps = []
    for i in range(NCORES):
        in_maps.append({
            "b_target": shard(inputs["B_target"], i),
            "b_infected": shard(inputs["B_infected"], i),
            "e_target": shard(inputs["E_target"], i),
            "e_infected": shard(inputs["E_infected"], i),
            "c_target": shard(inputs["C_target"], i),
            "c_infected": shard(inputs["C_infected"], i),
            "w1": np.ascontiguousarray(inputs["W1"]),
            "w2": np.ascontiguousarray(inputs["W2"]),
            "gamma_bs": np.ascontiguousarray(inputs["gamma_bs"]),
            "beta_bs": np.ascontiguousarray(inputs["beta_bs"]),
            "gamma_cs": np.ascontiguousarray(inputs["gamma_cs"]),
            "beta_cs": np.ascontiguousarray(inputs["beta_cs"]),
        })
    res = run_bass_kernel_spmd(nc, in_maps, list(range(NCORES)), trace=TRACE)
    LAST_RESULT.clear()
    LAST_RESULT.append(res)
    out = np.concatenate([r["o_out"] for r in res.results], axis=0)
    bs = np.concatenate([r["o_bs"] for r in res.results], axis=0)
    cs = np.concatenate([r["o_cs"] for r in res.results], axis=0)
    return (out, bs, cs)


def bench(iters=32, **inputs):
    """Amortized real-HW timing: pipelined repeated NEFF executions with
    inputs resident on device. Returns (per_iter_seconds, results_list)."""
    import time
    import jax
    from jax.sharding import Mesh, PartitionSpec, NamedSharding
    from jax.experimental.shard_map import shard_map
    from concourse import bass2jax
    from concourse import mybir as _mb

    if not _NC_CACHE:
        _NC_CACHE.append(build())
    nc = _NC_CACHE[0]
    bass2jax.install_neuronx_cc_hook()

    key_map = {
        "b_target": "B_target", "b_infected": "B_infected",
        "e_target": "E_target", "e_infected": "E_infected",
        "c_target": "C_target", "c_infected": "C_infected",
        "w1": "W1", "w2": "W2", "gamma_bs": "gamma_bs",
        "beta_bs": "beta_bs", "gamma_cs": "gamma_cs", "beta_cs": "beta_cs",
    }
    partition_name = (nc.partition_id_tensor.name
                      if nc.partition_id_tensor else None)
    in_names, out_names, out_avals, zero_outs = [], [], [], []
    for alloc in nc.m.functions[0].allocations:
        if not isinstance(alloc, _mb.MemoryLocationSet):
            continue
        name = alloc.memorylocations[0].name
        if alloc.kind == "ExternalInput" and name != partition_name:
            in_names.append(name)
        elif alloc.kind == "ExternalOutput":
            out_names.append(name)
            shp, dt = tuple(alloc.tensor_shape), _mb.dt.np(alloc.dtype)
            out_avals.append(jax.core.ShapedArray(shp, dt))
            zero_outs.append(np.zeros(shp, dt))
    n_params = len(in_names)
    all_names = in_names + out_names
    if partition_name is not None:
        all_names.append(partition_name)

    def _body(*args):
        operands = list(args)
        if partition_name is not None:
            operands.append(bass2jax.partition_id_tensor())
        return tuple(bass2jax._bass_exec_p.bind(
            *operands,
            out_avals=tuple(out_avals),
            in_names=tuple(all_names),
            out_names=tuple(out_names),
            lowering_input_output_aliases=(),
            sim_require_finite=True,
            sim_require_nnan=True,
            nc=nc,
        ))

    devices = jax.devices()[:NCORES]
    mesh = Mesh(np.asarray(devices), ("core",))
    n_outs = len(out_names)
    donate = tuple(range(n_params, n_params + n_outs))
    sharded = jax.jit(
        shard_map(_body, mesh=mesh,
                  in_specs=(PartitionSpec("core"),) * (n_params + n_outs),
                  out_specs=(PartitionSpec("core"),) * n_outs,
                  check_rep=False),
        donate_argnums=donate, keep_unused=True)

    concat_in = []
    for n in in_names:
        full = np.asarray(inputs[key_map[n]], np.float32)
        if key_map[n] in ("B_target", "B_infected", "E_target", "E_infected",
                          "C_target", "C_infected"):
            concat_in.append(np.ascontiguousarray(full))
        else:
            concat_in.append(np.concatenate([full] * NCORES, axis=0))
    sh = NamedSharding(mesh, PartitionSpec("core"))
    concat_in_dev = [jax.device_put(x, sh) for x in concat_in]
    concat_zeros = [np.zeros((NCORES * z.shape[0], *z.shape[1:]), z.dtype)
                    for z in zero_outs]

    outs = sharded(*concat_in_dev, *[np.copy(z) for z in concat_zeros])
    jax.block_until_ready(outs)
    outs = sharded(*concat_in_dev, *[np.copy(z) for z in concat_zeros])
    jax.block_until_ready(outs)
    zsets = [[jax.device_put(z, sh) for z in concat_zeros]
             for _ in range(iters)]
    for zs in zsets:
        jax.block_until_ready(zs)
    import time as _t
    t0 = _t.perf_counter()
    last = None
    for zs in zsets:
        last = sharded(*concat_in_dev, *zs)
    jax.block_until_ready(last)
    t1 = _t.perf_counter()
    return (t1 - t0) / iters, [np.asarray(o) for o in last]

